# revision 16
# baseline (speedup 1.0000x reference)
"""Trainium2 Bass kernel for nn_DCCEngine (cluster-attention pooling block).

Reference computation per batch b:
  sim   = x_flat @ centers.T * C^-0.5        [N,K]   (N=16384 pixels, K=64)
  attn  = softmax(sim, -1)
  cluster = attn.T @ x_flat                  [K,C]
  refined = silu(dwconv7x7(cluster.T as [C,8,8]) + dw_b)
  out   = attn @ refined_flat                [N,C]
  y     = pw_w @ out + pw_b
  result = x + group_norm(y) * gn_g + gn_b

Sharding: pure data-parallel, batch b -> core b (8 cores).

Structure (v2 — single-shot-latency optimized):
  - softmax without max-subtraction (sim ~ N(0, 0.02^2): exp is safe)
  - simp and srep share one PSUM bank: srep = [0 | J] @ expt writes the
    full [128,512] tile (rows 0:64 zeroed after exp consumed them)
  - phase A software-pipelined on PE: attn-transposes lag 2 chunks and
    gram matmuls lag 3 chunks behind sim/srep so the exp/recip/mul chain
    never stalls the in-order PE queue
  - GroupNorm stats computed analytically from the attention Gram matrix
    G = attn^T attn and a = attn^T 1 (no pass over y)
  - GN scale AND the per-channel constant (gnb + ach*(pwb-mean)) are both
    folded into lw (softmax columns sum to 1, so a constant row offset in
    lw adds const[o] to every pixel)
  - phase C does the residual add ON THE PE: up = lw'^T@attn + I^T@x in
    PSUM, one PSUM->SBUF copy (round-robin DVE/Act/Pool), then DMA out
  - dwconv taps split across DVE and Pool with independent partial
    accumulators (breaks the 49-op serial chain)
  - fp32 bits flow through PE in float32r mode for all big matmuls
"""
import numpy as np

import concourse.bass as bass
import concourse.tile as tile
from concourse import bacc, mybir
from concourse.bass_utils import run_bass_kernel_spmd

f32 = mybir.dt.float32
f32r = mybir.dt.float32r

C = 256
CH = 128          # channel half
N = 16384         # pixels per batch
K = 64            # clusters
CHUNK = 512
NCHUNK = N // CHUNK          # 32
GRP = 128                    # pixel group (cluster lhsT tile)
NGRP = CHUNK // GRP          # 4
GROUPS = 32
GSZ = C // GROUPS            # 8 channels per group
EPS = 1e-5

Exp = mybir.ActivationFunctionType.Exp
Sigmoid = mybir.ActivationFunctionType.Sigmoid
Sqrt = mybir.ActivationFunctionType.Sqrt
Copy = mybir.ActivationFunctionType.Copy
MUL = mybir.AluOpType.mult
ADD = mybir.AluOpType.add
DIV = mybir.AluOpType.divide

ATT_LAG = 3      # attn-transpose lag (chunks) behind sim on PE
GRAM_LAG = 3     # gram-matmul lag


def build_nc(repeat: int = 1, sim_f32r: bool = True, use_div: bool = False,
             xdma_cols: int = 2048):
    nc = bacc.Bacc("TRN2", target_bir_lowering=False, debug=False)

    def din(name, shape, dt_=f32):
        return nc.dram_tensor(name, list(shape), dt_, kind="ExternalInput").ap()

    x_d = din("x", [C, N])
    cenT_d = din("cenT", [C, K])       # (centers * C^-0.5).T
    ident_d = din("ident", [CH, CH])
    j64p_d = din("j64p", [K, CH])      # [0 | ones(64,64)]
    ones_d = din("ones", [CH, 2])
    dwt_d = din("dwt", [C, 49])
    dwb_d = din("dwb", [C, 1])
    pwb_d = din("pwb", [C, 1])
    gng_d = din("gng", [C, 1])
    gnb_d = din("gnb", [C, 1])
    pwbN_d = din("pwbN", [C, 1])       # N * pw_b
    npwb2_d = din("npwb2", [C, 1])     # N * pw_b^2
    pwb2_d = din("pwb2", [C, 1])       # 2 * pw_b
    pwT_d = din("pwT", [C, C])         # pw_w.T  ([c, o])
    gind_d = din("gind", [CH, 16])     # channel -> group (within half)
    gindT_d = din("gindT", [16, CH])
    out_d = nc.dram_tensor("out", [C, N], f32, kind="ExternalOutput").ap()

    simdt = f32r if sim_f32r else f32

    def half(ap_, h):
        return ap_[h * CH:(h + 1) * CH, :]

    with tile.TileContext(nc) as tc:
        with (
            tc.tile_pool(name="const", bufs=1) as cp,
            tc.tile_pool(name="xp", bufs=1) as xp,
            tc.tile_pool(name="apool", bufs=1) as apool,
            tc.tile_pool(name="wk", bufs=2) as wk,
            tc.tile_pool(name="stg", bufs=3) as stg,
            tc.tile_pool(name="wk2", bufs=4) as wk2,
            tc.tile_pool(name="wb", bufs=1) as wb,
        ):
            # ---- constants (all split into channel halves where C-sized) ----
            cenT, dwt, dwb, pwb, gng, gnb, pwbN, npwb2, pwb2, pwTh = (
                [], [], [], [], [], [], [], [], [], [])
            for h in range(2):
                t = cp.tile([CH, K], simdt, name=f"cenT{h}")
                nc.sync.dma_start(
                    t[:], half(cenT_d.bitcast(simdt) if sim_f32r else cenT_d, h))
                cenT.append(t)
                t = cp.tile([CH, 49], f32, name=f"dwt{h}")
                nc.sync.dma_start(t[:], half(dwt_d, h)); dwt.append(t)
                for lst, src in ((dwb, dwb_d), (pwb, pwb_d), (gng, gng_d),
                                 (gnb, gnb_d), (pwbN, pwbN_d),
                                 (npwb2, npwb2_d), (pwb2, pwb2_d)):
                    t = cp.tile([CH, 1], f32, name=f"c{len(lst)}_{id(src) % 997}_{h}")
                    nc.sync.dma_start(t[:], half(src, h)); lst.append(t)
                t = cp.tile([CH, C], f32, name=f"pwT{h}")
                nc.sync.dma_start(t[:], half(pwT_d, h)); pwTh.append(t)
            ident = cp.tile([CH, CH], f32, name="ident")
            nc.sync.dma_start(ident[:], ident_d)
            identr = cp.tile([CH, CH], f32r, name="identr")
            nc.sync.dma_start(identr[:], ident_d.bitcast(f32r))
            identr_hi = cp.tile([CH, K], f32r, name="identr_hi")
            nc.sync.dma_start(identr_hi[K:CH, :], ident_d.bitcast(f32r)[0:K, 0:K])
            j64p = cp.tile([K, CH], f32r, name="j64p")
            nc.sync.dma_start(j64p[:], j64p_d.bitcast(f32r))
            ones = cp.tile([CH, 2], f32r, name="ones")
            nc.sync.dma_start(ones[:], ones_d.bitcast(f32r))
            gind = cp.tile([CH, 16], f32, name="gind")
            nc.sync.dma_start(gind[:], gind_d)
            gindT = cp.tile([16, CH], f32, name="gindT")
            nc.sync.dma_start(gindT[:], gindT_d)

            # ---- persistent big buffers ----
            xh = []
            for h in range(2):
                t = xp.tile([CH, N], f32r, name=f"x{h}")
                xh.append(t)
            # attnT (k-major, normalized): chunks 0..15 -> rows 0:64,
            # chunks 16..31 -> rows 64:128.
            attnT = apool.tile([CH, N // 2], f32r, name="attnT")

            for rep in range(repeat):
                # big x-in DMAs (xdma_cols per transfer)
                for g in range(N // xdma_cols):
                    cs = g * xdma_cols
                    for h in range(2):
                        nc.sync.dma_start(
                            xh[h][:, cs:cs + xdma_cols],
                            x_d.bitcast(f32r)[h * CH:(h + 1) * CH,
                                              cs:cs + xdma_cols])

                # ---------- phase A (software-pipelined on PE) ----------
                with (
                    tc.tile_pool(name="psA", bufs=2, space="PSUM") as psA,
                    tc.tile_pool(name="psT", bufs=1, space="PSUM") as psT,
                    tc.tile_pool(name="psX", bufs=2, space="PSUM") as psX,
                    tc.tile_pool(name="psC", bufs=1, space="PSUM") as psC,
                ):
                    clp = psC.tile([K, 322], f32, name="clp", tag="clp")
                    ss = {}      # ch -> simp/srep combined psum tile
                    expts = {}   # ch -> expt sbuf tile
                    stages = {}  # ch -> stage sbuf tile

                    def rowhalf(ch):
                        return (0 if ch < 16 else K), (ch % 16) * CHUNK

                    def do_attt(ch):
                        """attn transposes + att stage copy for chunk ch."""
                        rh, ac = rowhalf(ch)
                        att_ps = psT.tile([CH, NGRP * K], f32r, name="att_ps",
                                          tag="att_ps")
                        idk = identr[0:K, 0:K] if rh == 0 else identr_hi[K:CH, :]
                        for g in range(NGRP):
                            nc.tensor.transpose(
                                att_ps[:, g * K:(g + 1) * K],
                                attnT[rh:rh + K,
                                      ac + g * GRP:ac + (g + 1) * GRP], idk)
                        stage4 = stages[ch][:].rearrange("p (g c) -> p g c",
                                                         g=NGRP)
                        nc.scalar.activation(
                            stage4[:, :, 256:320],
                            att_ps[:].rearrange("p (g c) -> p g c", g=NGRP),
                            Copy)

                    def do_front_pe(ch):
                        """xt transposes + sim matmuls for chunk ch (PE)."""
                        cs = ch * CHUNK
                        xt_ps = psX.tile([CH, 1024], f32r, name="xt_ps",
                                         tag="xt_ps")
                        for g in range(NGRP):
                            ps = cs + g * GRP
                            for h in range(2):
                                nc.tensor.transpose(
                                    xt_ps[:, g * 256 + h * CH:
                                          g * 256 + (h + 1) * CH],
                                    xh[h][:, ps:ps + GRP], identr[:])
                        sst = psA.tile([CH, CHUNK], f32, name="ss", tag="ss")
                        nc.tensor.matmul(sst[0:K, :], cenT[0][:],
                                         xh[0][:, cs:cs + CHUNK],
                                         start=True, stop=False)
                        nc.tensor.matmul(sst[0:K, :], cenT[1][:],
                                         xh[1][:, cs:cs + CHUNK],
                                         start=False, stop=True)
                        ss[ch] = sst
                        return xt_ps

                    def do_front_copies(ch, xt_ps):
                        """stage-x PSUM->SBUF copies (DVE half + Act half)."""
                        stage = stg.tile([CH, NGRP * 322], f32r, name="stage",
                                         tag="stage")
                        stage4 = stage[:].rearrange("p (g c) -> p g c", g=NGRP)
                        xt4 = xt_ps[:].rearrange("p (g c) -> p g c", g=NGRP)
                        nc.vector.tensor_copy(stage4[:, 0:2, 0:256],
                                              xt4[:, 0:2, :])
                        nc.vector.tensor_copy(
                            stage4[:, :, 320:322],
                            ones[:].unsqueeze(1).broadcast_to([CH, NGRP, 2]))
                        nc.scalar.activation(stage4[:, 2:4, 0:256],
                                             xt4[:, 2:4, :], Copy)
                        stages[ch] = stage

                    def do_srep(ch):
                        """srep matmul + recip + normalize-mul for chunk ch."""
                        sst = ss[ch]
                        rh, ac = rowhalf(ch)
                        expt = expts[ch]
                        # srep = [0|J] @ expt -> rows 64:128 (rows 0:64 zeroed)
                        nc.tensor.matmul(sst[:], j64p[:], expt[:],
                                         start=True, stop=True,
                                         skip_group_check=True)
                        if use_div:
                            nc.vector.tensor_tensor(
                                attnT[rh:rh + K, ac:ac + CHUNK],
                                expt[:].bitcast(f32), sst[K:CH, :], DIV)
                        else:
                            rs = wk.tile([K, CHUNK], f32, name="rs", tag="rs")
                            nc.vector.reciprocal(rs[:], sst[K:CH, :])
                            nc.gpsimd.tensor_mul(
                                attnT[rh:rh + K, ac:ac + CHUNK],
                                expt[:].bitcast(f32), rs[:])

                    def do_exp(ch):
                        expt = wk.tile([K, CHUNK], f32r, name="expt",
                                       tag="expt")
                        nc.scalar.activation(expt[:], ss[ch][0:K, :], Exp)
                        expts[ch] = expt

                    def do_gram(ch):
                        stage = stages.pop(ch)
                        for g in range(NGRP):
                            first = (ch == 0 and g == 0)
                            last = (ch == NCHUNK - 1 and g == NGRP - 1)
                            nc.tensor.matmul(clp[:],
                                             stage[:, g * 322 + 256:
                                                   g * 322 + 320],
                                             stage[:, g * 322:g * 322 + 322],
                                             start=first, stop=last,
                                             skip_group_check=True)
                        ss.pop(ch, None); expts.pop(ch, None)

                    for ch in range(NCHUNK + GRAM_LAG):
                        if ATT_LAG <= ch < NCHUNK + ATT_LAG:
                            do_attt(ch - ATT_LAG)
                        if 1 <= ch < NCHUNK + 1:
                            do_srep(ch - 1)
                        xt_ps = None
                        if ch < NCHUNK:
                            xt_ps = do_front_pe(ch)
                            do_exp(ch)
                        if ch >= GRAM_LAG:
                            do_gram(ch - GRAM_LAG)
                        if ch < NCHUNK:
                            do_front_copies(ch, xt_ps)

                    cl_sb = wb.tile([K, 322], f32, name="cl_sb")
                    nc.vector.tensor_copy(cl_sb[:], clp[:])

                # ---------- phase B ----------
                with tc.tile_pool(name="psB", bufs=1, space="PSUM") as psB:
                    grid = []
                    for h in range(2):
                        gp = psB.tile([CH, K], f32, name=f"gp{h}", tag="gp")
                        nc.tensor.transpose(gp[:], cl_sb[:, h * CH:(h + 1) * CH],
                                            ident[0:K, 0:K])
                        gsb = wb.tile([CH, K], f32, name=f"grid{h}")
                        nc.vector.tensor_copy(gsb[:], gp[:])
                        grid.append(gsb)
                    pads_v, accs, accs_v = [], [], []
                    for h in range(2):
                        pad = wb.tile([CH, 196], f32, name=f"pad{h}")
                        nc.vector.memset(pad[:], 0.0)
                        padv = pad[:].rearrange("p (r c) -> p r c", r=14)
                        nc.vector.tensor_copy(
                            padv[:, 3:11, 3:11],
                            grid[h][:].rearrange("p (r c) -> p r c", r=8))
                        pads_v.append(padv)
                        acc = wb.tile([CH, K], f32, name=f"racc{h}")
                        accs.append(acc)
                        accs_v.append(acc[:].rearrange("p (r c) -> p r c", r=8))
                    # interleave halves per tap: consecutive DVE ops touch
                    # different accumulators, keeping the pipeline full
                    for t in range(49):
                        dr, dc = t // 7, t % 7
                        for h in range(2):
                            win = pads_v[h][:, dr:dr + 8, dc:dc + 8]
                            tap = dwt[h][:, t:t + 1]
                            if t == 0:
                                nc.vector.tensor_scalar_mul(accs_v[h], win,
                                                            tap)
                            else:
                                nc.vector.scalar_tensor_tensor(
                                    accs_v[h], win, tap, accs_v[h],
                                    op0=MUL, op1=ADD)
                    refined = []
                    for h in range(2):
                        acc = accs[h]
                        # silu(z) = z * sigmoid(z), z = acc + dw_b
                        sg = wb.tile([CH, K], f32, name=f"sg{h}")
                        nc.scalar.activation(sg[:], acc[:], Sigmoid,
                                             bias=dwb[h][:])
                        zt = wb.tile([CH, K], f32, name=f"zt{h}")
                        nc.vector.tensor_scalar_add(zt[:], acc[:], dwb[h][:])
                        nc.vector.tensor_mul(acc[:], zt[:], sg[:])
                        refined.append(acc)
                    wrt = []
                    for oh in range(2):
                        wp = psB.tile([CH, K], f32, name=f"wp{oh}", tag="wp")
                        for h in range(2):
                            nc.tensor.matmul(
                                wp[:], pwTh[h][:, oh * CH:(oh + 1) * CH],
                                refined[h][:], start=(h == 0), stop=(h == 1))
                        wsb = wb.tile([CH, K], f32, name=f"wrt{oh}")
                        nc.vector.tensor_copy(wsb[:], wp[:])
                        wrt.append(wsb)
                    wrtt = wb.tile([K, C], f32, name="wrtt")
                    for oh in range(2):
                        tp = psB.tile([K, CH], f32, name=f"tp{oh}", tag="tp")
                        nc.tensor.transpose(tp[:], wrt[oh][:], ident[:])
                        nc.vector.tensor_copy(wrtt[:, oh * CH:(oh + 1) * CH],
                                              tp[:])
                    a_col = cl_sb[:, 320:321]
                    g_mat = cl_sb[:, 256:320]
                    stats = []
                    for oh in range(2):
                        st = wb.tile([CH, 2], f32, name=f"stats{oh}")
                        wa = psB.tile([CH, 1], f32, name=f"wa{oh}", tag="wa")
                        nc.tensor.matmul(wa[:], wrtt[:, oh * CH:(oh + 1) * CH],
                                         a_col, start=True, stop=True)
                        nc.vector.scalar_tensor_tensor(
                            st[:, 0:1], wa[:], 1.0, pwbN[oh][:],
                            op0=MUL, op1=ADD)
                        qp = psB.tile([CH, K], f32, name=f"qp{oh}", tag="qp")
                        nc.tensor.matmul(qp[:], wrtt[:, oh * CH:(oh + 1) * CH],
                                         g_mat, start=True, stop=True)
                        scr = wb.tile([CH, K], f32, name=f"scr{oh}")
                        quad = wb.tile([CH, 1], f32, name=f"quad{oh}")
                        nc.vector.tensor_mul(scr[:], qp[:], wrt[oh][:])
                        nc.vector.reduce_sum(quad[:], scr[:],
                                             axis=mybir.AxisListType.X)
                        t2 = wb.tile([CH, 1], f32, name=f"t2{oh}")
                        nc.vector.scalar_tensor_tensor(
                            t2[:], wa[:], pwb2[oh][:], npwb2[oh][:],
                            op0=MUL, op1=ADD)
                        nc.vector.tensor_add(st[:, 1:2], t2[:], quad[:])
                        stats.append(st)
                    gs = wb.tile([16, 4], f32, name="gs")
                    for oh in range(2):
                        gp2 = psB.tile([16, 2], f32, name=f"gp2{oh}", tag="gp2")
                        nc.tensor.matmul(gp2[:], gind[:], stats[oh][:],
                                         start=True, stop=True)
                        nc.vector.tensor_copy(gs[:, oh * 2:(oh + 1) * 2], gp2[:])
                    gs4 = gs[:].rearrange("p (h c) -> p h c", h=2)
                    mv = wb.tile([16, 4], f32, name="mv")  # [mean, rstd] x half
                    mv4 = mv[:].rearrange("p (h c) -> p h c", h=2)
                    cinv = 1.0 / (GSZ * N)
                    nc.vector.tensor_scalar_mul(mv4[:, :, 0:1], gs4[:, :, 0:1],
                                                cinv)
                    ex2 = wb.tile([16, 2], f32, name="ex2")
                    nc.vector.tensor_scalar_mul(ex2[:], gs4[:, :, 1], cinv)
                    m2 = wb.tile([16, 2], f32, name="m2")
                    nc.vector.tensor_mul(m2[:], mv4[:, :, 0], mv4[:, :, 0])
                    var = wb.tile([16, 2], f32, name="var")
                    nc.vector.tensor_sub(var[:], ex2[:], m2[:])
                    epst = wb.tile([16, 1], f32, name="epst")
                    nc.vector.memset(epst[:], EPS)
                    std = wb.tile([16, 2], f32, name="std")
                    nc.scalar.activation(std[:], var[:], Sqrt, bias=epst[:])
                    nc.vector.reciprocal(mv4[:, :, 1], std[:])
                    lw = wb.tile([CH, C], f32r, name="lw")
                    for oh in range(2):
                        ep = psB.tile([CH, 2], f32, name=f"ep{oh}", tag="ep")
                        nc.tensor.matmul(ep[:], gindT[:],
                                         mv[:, oh * 2:(oh + 1) * 2],
                                         start=True, stop=True)
                        ach = wb.tile([CH, 1], f32, name=f"ach{oh}")
                        nc.vector.tensor_mul(ach[:], gng[oh][:], ep[:, 1:2])
                        cst = wb.tile([CH, 1], f32, name=f"cst{oh}")
                        nc.vector.tensor_sub(cst[:], pwb[oh][:], ep[:, 0:1])
                        nc.vector.tensor_mul(cst[:], ach[:], cst[:])
                        nc.vector.tensor_add(cst[:], gnb[oh][:], cst[:])
                        lwp = wb.tile([CH, K], f32, name=f"lwp{oh}")
                        nc.vector.tensor_scalar_mul(lwp[:], wrt[oh][:], ach[:])
                        # fold the per-channel const into lw: softmax columns
                        # sum to 1, so lw'[k,o] = lw[k,o] + const[o] adds
                        # const[o] to every output pixel
                        nc.vector.tensor_scalar_add(lwp[:], lwp[:], cst[:])
                        ltp = psB.tile([K, CH], f32, name=f"ltp{oh}", tag="ltp")
                        nc.tensor.transpose(ltp[:], lwp[:], ident[:])
                        nc.scalar.activation(lw[0:K, oh * CH:(oh + 1) * CH],
                                             ltp[:], Copy)
                        nc.scalar.activation(lw[K:CH, oh * CH:(oh + 1) * CH],
                                             ltp[:], Copy)

                # ---------- phase C ----------
                with tc.tile_pool(name="psU", bufs=8, space="PSUM") as psU:
                    rr = 0
                    for ch in range(NCHUNK):
                        cs = ch * CHUNK
                        rh = 0 if ch < 16 else K
                        ac = (ch % 16) * CHUNK
                        for oh in range(2):
                            up = psU.tile([CH, CHUNK], f32, name="up", tag="up")
                            nc.tensor.matmul(up[:],
                                             lw[rh:rh + K, oh * CH:(oh + 1) * CH],
                                             attnT[rh:rh + K, ac:ac + CHUNK],
                                             start=True, stop=False)
                            nc.tensor.matmul(up[:], identr[:],
                                             xh[oh][:, cs:cs + CHUNK],
                                             start=False, stop=True)
                            osb = wk2.tile([CH, CHUNK], f32, name="osb",
                                           tag="osb")
                            if rr % 2 == 0:
                                nc.vector.tensor_copy(osb[:], up[:])
                            else:
                                nc.scalar.activation(osb[:], up[:], Copy)
                            rr += 1
                            nc.sync.dma_start(
                                out_d[oh * CH:(oh + 1) * CH, cs:cs + CHUNK],
                                osb[:])
    nc.compile()
    return nc


def host_prep(centers, dw_w, dw_b, pw_w, pw_b, gn_g, gn_b):
    cenT = np.ascontiguousarray((centers * (C ** -0.5)).T.astype(np.float32))
    col = lambda v: np.ascontiguousarray(
        np.asarray(v, dtype=np.float32).reshape(C, 1))
    gind = np.zeros((CH, 16), dtype=np.float32)
    for c in range(CH):
        gind[c, c // GSZ] = 1.0
    j64p = np.zeros((K, CH), dtype=np.float32)
    j64p[:, K:CH] = 1.0
    return {
        "cenT": cenT,
        "ident": np.eye(CH, dtype=np.float32),
        "j64p": j64p,
        "ones": np.ones((CH, 2), dtype=np.float32),
        "dwt": np.ascontiguousarray(
            np.asarray(dw_w, dtype=np.float32).reshape(C, 49)),
        "dwb": col(dw_b), "pwb": col(pw_b), "gng": col(gn_g), "gnb": col(gn_b),
        "pwbN": col(np.asarray(pw_b) * float(N)),
        "npwb2": col(np.asarray(pw_b) * np.asarray(pw_b) * float(N)),
        "pwb2": col(2.0 * np.asarray(pw_b)),
        "pwT": np.ascontiguousarray(np.asarray(pw_w, dtype=np.float32).T),
        "gind": gind,
        "gindT": np.ascontiguousarray(gind.T),
    }


_NC_CACHE = {}


def _get_nc(repeat=1, sim_f32r=True, use_div=False):
    key = (repeat, sim_f32r, use_div)
    if key not in _NC_CACHE:
        _NC_CACHE[key] = build_nc(repeat=repeat, sim_f32r=sim_f32r,
                                  use_div=use_div)
    return _NC_CACHE[key]


def kernel(x, centers, dw_w, dw_b, pw_w, pw_b, gn_g, gn_b,
           repeat=1, sim_f32r=True, use_div=False):
    x = np.asarray(x)
    B = x.shape[0]
    nc = _get_nc(repeat=repeat, sim_f32r=sim_f32r, use_div=use_div)
    consts = host_prep(np.asarray(centers), np.asarray(dw_w), np.asarray(dw_b),
                       np.asarray(pw_w), np.asarray(pw_b),
                       np.asarray(gn_g), np.asarray(gn_b))
    in_maps = []
    for b in range(B):
        m = dict(consts)
        m["x"] = np.ascontiguousarray(x[b].reshape(C, N).astype(np.float32))
        in_maps.append(m)
    res = run_bass_kernel_spmd(nc, in_maps, core_ids=list(range(B)))
    out = np.stack([r["out"].reshape(C, 128, 128) for r in res.results])
    return out.astype(np.float32)


# revision 29
# speedup vs baseline: 1.1473x; 1.1473x over previous
"""Trainium2 Bass kernel for nn_DCCEngine (cluster-attention pooling block).

Reference computation per batch b:
  sim   = x_flat @ centers.T * C^-0.5        [N,K]   (N=16384 pixels, K=64)
  attn  = softmax(sim, -1)
  cluster = attn.T @ x_flat                  [K,C]
  refined = silu(dwconv7x7(cluster.T as [C,8,8]) + dw_b)
  out   = attn @ refined_flat                [N,C]
  y     = pw_w @ out + pw_b
  result = x + group_norm(y) * gn_g + gn_b

Sharding: pure data-parallel, batch b -> core b (8 cores).

Structure (v2 — single-shot-latency optimized):
  - softmax without max-subtraction (sim ~ N(0, 0.02^2): exp is safe)
  - simp and srep share one PSUM bank: srep = [0 | J] @ expt writes the
    full [128,512] tile (rows 0:64 zeroed after exp consumed them)
  - phase A software-pipelined on PE: attn-transposes lag 2 chunks and
    gram matmuls lag 3 chunks behind sim/srep so the exp/recip/mul chain
    never stalls the in-order PE queue
  - GroupNorm stats computed analytically from the attention Gram matrix
    G = attn^T attn and a = attn^T 1 (no pass over y)
  - GN scale AND the per-channel constant (gnb + ach*(pwb-mean)) are both
    folded into lw (softmax columns sum to 1, so a constant row offset in
    lw adds const[o] to every pixel)
  - phase C does the residual add ON THE PE: up = lw'^T@attn + I^T@x in
    PSUM, one PSUM->SBUF copy (round-robin DVE/Act/Pool), then DMA out
  - dwconv taps split across DVE and Pool with independent partial
    accumulators (breaks the 49-op serial chain)
  - fp32 bits flow through PE in float32r mode for all big matmuls
"""
import numpy as np

import concourse.bass as bass
import concourse.tile as tile
from concourse import bacc, mybir
from concourse.bass_utils import run_bass_kernel_spmd

f32 = mybir.dt.float32
f32r = mybir.dt.float32r

C = 256
CH = 128          # channel half
N = 16384         # pixels per batch
K = 64            # clusters
CHUNK = 512
NCHUNK = N // CHUNK          # 32
GRP = 128                    # pixel group (cluster lhsT tile)
NGRP = CHUNK // GRP          # 4
GROUPS = 32
GSZ = C // GROUPS            # 8 channels per group
EPS = 1e-5

Exp = mybir.ActivationFunctionType.Exp
Sigmoid = mybir.ActivationFunctionType.Sigmoid
Sqrt = mybir.ActivationFunctionType.Sqrt
Copy = mybir.ActivationFunctionType.Copy
MUL = mybir.AluOpType.mult
ADD = mybir.AluOpType.add
DIV = mybir.AluOpType.divide

ATT_LAG = 3      # attn-transpose lag (chunks) behind sim on PE
GRAM_LAG = 3     # gram-matmul lag


def build_nc(repeat: int = 1, sim_f32r: bool = True, use_div: bool = False,
             xdma_cols: int = 2048):
    nc = bacc.Bacc("TRN2", target_bir_lowering=False, debug=False)

    def din(name, shape, dt_=f32):
        return nc.dram_tensor(name, list(shape), dt_, kind="ExternalInput").ap()

    x_d = din("x", [C, N])
    # cpack [C, 376]: per-channel consts, loaded in one DMA.
    #   [0:64 cenT | 64:113 dwt | 113:120 col-consts | 120:376 pwT]
    # col-consts order: dwb, pwb, gng, gnb, pwbN, npwb2, pwb2
    cpack_d = din("cpack", [C, 376])
    # mpack [128, 466]: partition-shaped consts, one DMA.
    #   [0:128 ident | 128:192 identr_hi | 192:320 j64p | 320:322 ones
    #    | 322:338 gind | 338:466 gindT(rows 0:16)]
    mpack_d = din("mpack", [CH, 466])
    out_d = nc.dram_tensor("out", [C, N], f32, kind="ExternalOutput").ap()

    simdt = f32r if sim_f32r else f32

    def half(ap_, h):
        return ap_[h * CH:(h + 1) * CH, :]

    with tile.TileContext(nc) as tc:
        with (
            tc.tile_pool(name="const", bufs=1) as cp,
            tc.tile_pool(name="xp", bufs=1) as xp,
            tc.tile_pool(name="apool", bufs=1) as apool,
            tc.tile_pool(name="wk", bufs=2) as wk,
            tc.tile_pool(name="stg", bufs=3) as stg,
            tc.tile_pool(name="wk2", bufs=3) as wk2,
            tc.tile_pool(name="wb", bufs=1) as wb,
        ):
            # ---- persistent big buffers ----
            xh = []
            for h in range(2):
                t = xp.tile([CH, N], f32r, name=f"x{h}")
                xh.append(t)
            # attnT (k-major, normalized): chunks 0..15 -> rows 0:64,
            # chunks 16..31 -> rows 64:128.
            attnT = apool.tile([CH, N // 2], f32r, name="attnT")

            # ---- constants: two packed DMAs (issued after first x slices) ----
            cpk = cp.tile([CH, 2 * 376], simdt, name="cpk")
            mpk = cp.tile([CH, 466], f32r, name="mpk")

            def cenT(h):
                return cpk[:, h * 376:h * 376 + K]

            def dwt_tap(h, t):
                return cpk[:, h * 376 + 64 + t:
                           h * 376 + 64 + t + 1].bitcast(f32)

            def dwt_blk(h):
                return cpk[:, h * 376 + 64:h * 376 + 113].bitcast(f32)

            def ccol(h, idx):
                return cpk[:, h * 376 + 113 + idx:
                           h * 376 + 113 + idx + 1].bitcast(f32)

            dwb = lambda h: ccol(h, 0)
            pwb = lambda h: ccol(h, 1)
            gng = lambda h: ccol(h, 2)
            gnb = lambda h: ccol(h, 3)
            pwbN = lambda h: ccol(h, 4)
            npwb2 = lambda h: ccol(h, 5)
            pwb2 = lambda h: ccol(h, 6)

            def pwT_blk(h, oh):
                return cpk[:, h * 376 + 120 + oh * CH:
                           h * 376 + 120 + (oh + 1) * CH].bitcast(f32)

            def identf(p0=0, p1=CH, c0=0, c1=CH):
                return mpk[p0:p1, c0:c1].bitcast(f32)

            def identr_ap(p0=0, p1=CH, c0=0, c1=CH):
                return mpk[p0:p1, c0:c1]

            def identr_hi_ap():
                return mpk[K:CH, 128:192]

            def j64p_ap():
                return mpk[0:K, 192:320]

            def ones_ap():
                return mpk[:, 320:322]

            def gind_ap():
                return mpk[:, 322:338].bitcast(f32)

            def gindT_ap():
                return mpk[0:16, 338:466].bitcast(f32)

            first_load = True

            for rep in range(repeat):
                # big x-in DMAs (xdma_cols per transfer); consts ride after
                # the first pair so phase A can start almost immediately
                for g in range(N // xdma_cols):
                    cs = g * xdma_cols
                    for h in range(2):
                        nc.sync.dma_start(
                            xh[h][:, cs:cs + xdma_cols],
                            x_d.bitcast(f32r)[h * CH:(h + 1) * CH,
                                              cs:cs + xdma_cols])
                    if g == 0 and first_load:
                        first_load = False
                        nc.sync.dma_start(
                            cpk[:].rearrange("p (u n) -> p u n", u=2),
                            cpack_d.bitcast(simdt).rearrange(
                                "(u p) n -> p u n", u=2))
                        nc.sync.dma_start(mpk[:], mpack_d.bitcast(f32r))

                # ---------- phase A (software-pipelined on PE) ----------
                with (
                    tc.tile_pool(name="psA", bufs=2, space="PSUM") as psA,
                    tc.tile_pool(name="psT", bufs=1, space="PSUM") as psT,
                    tc.tile_pool(name="psX", bufs=2, space="PSUM") as psX,
                    tc.tile_pool(name="psC", bufs=1, space="PSUM") as psC,
                ):
                    clp = psC.tile([K, 322], f32, name="clp", tag="clp")
                    ss = {}      # ch -> simp/srep combined psum tile
                    expts = {}   # ch -> expt sbuf tile
                    stages = {}  # ch -> stage sbuf tile

                    def rowhalf(ch):
                        return (0 if ch < 16 else K), (ch % 16) * CHUNK

                    def do_attt(ch):
                        """attn transposes + att stage copy for chunk ch."""
                        rh, ac = rowhalf(ch)
                        att_ps = psT.tile([CH, NGRP * K], f32r, name="att_ps",
                                          tag="att_ps")
                        idk = identr_ap(0, K, 0, K) if rh == 0 else identr_hi_ap()
                        for g in range(NGRP):
                            nc.tensor.transpose(
                                att_ps[:, g * K:(g + 1) * K],
                                attnT[rh:rh + K,
                                      ac + g * GRP:ac + (g + 1) * GRP], idk)
                        stage4 = stages[ch][:].rearrange("p (g c) -> p g c",
                                                         g=NGRP)
                        nc.scalar.activation(
                            stage4[:, :, 256:320],
                            att_ps[:].rearrange("p (g c) -> p g c", g=NGRP),
                            Copy)

                    def do_front_pe(ch):
                        """xt transposes + sim matmuls for chunk ch (PE)."""
                        cs = ch * CHUNK
                        xt_ps = psX.tile([CH, 1024], f32r, name="xt_ps",
                                         tag="xt_ps")
                        for g in range(NGRP):
                            ps = cs + g * GRP
                            for h in range(2):
                                nc.tensor.transpose(
                                    xt_ps[:, g * 256 + h * CH:
                                          g * 256 + (h + 1) * CH],
                                    xh[h][:, ps:ps + GRP], identr_ap())
                        sst = psA.tile([CH, CHUNK], f32, name="ss", tag="ss")
                        nc.tensor.matmul(sst[0:K, :], cenT(0),
                                         xh[0][:, cs:cs + CHUNK],
                                         start=True, stop=False)
                        nc.tensor.matmul(sst[0:K, :], cenT(1),
                                         xh[1][:, cs:cs + CHUNK],
                                         start=False, stop=True)
                        ss[ch] = sst
                        return xt_ps

                    def do_front_copies(ch, xt_ps):
                        """stage-x PSUM->SBUF copies (DVE half + Act half)."""
                        stage = stg.tile([CH, NGRP * 322], f32r, name="stage",
                                         tag="stage")
                        stage4 = stage[:].rearrange("p (g c) -> p g c", g=NGRP)
                        xt4 = xt_ps[:].rearrange("p (g c) -> p g c", g=NGRP)
                        nc.vector.tensor_copy(stage4[:, 0:2, 0:256],
                                              xt4[:, 0:2, :])
                        nc.vector.tensor_copy(
                            stage4[:, :, 320:322],
                            ones_ap().unsqueeze(1).broadcast_to([CH, NGRP, 2]))
                        nc.scalar.activation(stage4[:, 2:4, 0:256],
                                             xt4[:, 2:4, :], Copy)
                        stages[ch] = stage

                    def do_srep(ch):
                        """srep matmul + recip + normalize-mul for chunk ch."""
                        sst = ss[ch]
                        rh, ac = rowhalf(ch)
                        expt = expts[ch]
                        # srep = [0|J] @ expt -> rows 64:128 (rows 0:64 zeroed)
                        nc.tensor.matmul(sst[:], j64p_ap(), expt[:],
                                         start=True, stop=True,
                                         skip_group_check=True)
                        if use_div:
                            nc.vector.tensor_tensor(
                                attnT[rh:rh + K, ac:ac + CHUNK],
                                expt[:].bitcast(f32), sst[K:CH, :], DIV)
                        else:
                            rs = wk.tile([K, CHUNK], f32, name="rs", tag="rs")
                            nc.vector.reciprocal(rs[:], sst[K:CH, :])
                            nc.gpsimd.tensor_mul(
                                attnT[rh:rh + K, ac:ac + CHUNK],
                                expt[:].bitcast(f32), rs[:])

                    def do_exp(ch):
                        expt = wk.tile([K, CHUNK], f32r, name="expt",
                                       tag="expt")
                        nc.scalar.activation(expt[:], ss[ch][0:K, :], Exp)
                        expts[ch] = expt

                    def do_gram(ch):
                        stage = stages.pop(ch)
                        for g in range(NGRP):
                            first = (ch == 0 and g == 0)
                            last = (ch == NCHUNK - 1 and g == NGRP - 1)
                            nc.tensor.matmul(clp[:],
                                             stage[:, g * 322 + 256:
                                                   g * 322 + 320],
                                             stage[:, g * 322:g * 322 + 322],
                                             start=first, stop=last,
                                             skip_group_check=True)
                        ss.pop(ch, None); expts.pop(ch, None)

                    for ch in range(NCHUNK + GRAM_LAG):
                        if ATT_LAG <= ch < NCHUNK + ATT_LAG:
                            do_attt(ch - ATT_LAG)
                        if 1 <= ch < NCHUNK + 1:
                            do_srep(ch - 1)
                        xt_ps = None
                        if ch < NCHUNK:
                            xt_ps = do_front_pe(ch)
                            do_exp(ch)
                        if ch >= GRAM_LAG:
                            do_gram(ch - GRAM_LAG)
                        if ch < NCHUNK:
                            do_front_copies(ch, xt_ps)

                    cl_sb = wb.tile([K, 322], f32, name="cl_sb")
                    nc.vector.tensor_copy(cl_sb[:], clp[:])

                # ---------- phase B ----------
                with tc.tile_pool(name="psB", bufs=1, space="PSUM") as psB:
                    pads_v, accs, accs_v = [], [], []
                    for h in range(2):
                        gp = psB.tile([CH, K], f32, name=f"gp{h}", tag="gp")
                        nc.tensor.transpose(gp[:], cl_sb[:, h * CH:(h + 1) * CH],
                                            identf(0, K, 0, K))
                        pad = wb.tile([CH, 196], f32, name=f"pad{h}")
                        nc.vector.memset(pad[:], 0.0)
                        padv = pad[:].rearrange("p (r c) -> p r c", r=14)
                        nc.vector.tensor_copy(
                            padv[:, 3:11, 3:11],
                            gp[:].rearrange("p (r c) -> p r c", r=8))
                        pads_v.append(padv)
                        acc = wb.tile([CH, K], f32, name=f"racc{h}")
                        accs.append(acc)
                        accs_v.append(acc[:].rearrange("p (r c) -> p r c", r=8))
                    # interleave halves per tap: consecutive DVE ops touch
                    # different accumulators, keeping the pipeline full
                    for t in range(49):
                        dr, dc = t // 7, t % 7
                        for h in range(2):
                            win = pads_v[h][:, dr:dr + 8, dc:dc + 8]
                            tap = dwt_tap(h, t)
                            if t == 0:
                                nc.vector.tensor_scalar_mul(accs_v[h], win,
                                                            tap)
                            else:
                                nc.vector.scalar_tensor_tensor(
                                    accs_v[h], win, tap, accs_v[h],
                                    op0=MUL, op1=ADD)
                    refined = []
                    for h in range(2):
                        acc = accs[h]
                        # silu(z) = z * sigmoid(z), z = acc + dw_b
                        sg = wb.tile([CH, K], f32, name="sg")
                        nc.scalar.activation(sg[:], acc[:], Sigmoid,
                                             bias=dwb(h))
                        zt = wb.tile([CH, K], f32, name="zt")
                        nc.vector.tensor_scalar_add(zt[:], acc[:], dwb(h))
                        nc.vector.tensor_mul(acc[:], zt[:], sg[:])
                        refined.append(acc)
                    wrt = []
                    for oh in range(2):
                        wp = psB.tile([CH, K], f32, name=f"wp{oh}", tag="wp")
                        for h in range(2):
                            nc.tensor.matmul(
                                wp[:], pwT_blk(h, oh),
                                refined[h][:], start=(h == 0), stop=(h == 1))
                        wsb = wb.tile([CH, K], f32, name=f"wrt{oh}")
                        nc.vector.tensor_copy(wsb[:], wp[:])
                        wrt.append(wsb)
                    wrtt = wb.tile([K, C], f32, name="wrtt")
                    for oh in range(2):
                        tp = psB.tile([K, CH], f32, name=f"tp{oh}", tag="tp")
                        nc.tensor.transpose(tp[:], wrt[oh][:], identf())
                        nc.vector.tensor_copy(wrtt[:, oh * CH:(oh + 1) * CH],
                                              tp[:])
                    a_col = cl_sb[:, 320:321]
                    g_mat = cl_sb[:, 256:320]
                    stats = []
                    for oh in range(2):
                        st = wb.tile([CH, 2], f32, name=f"stats{oh}")
                        wa = psB.tile([CH, 1], f32, name=f"wa{oh}", tag="wa")
                        nc.tensor.matmul(wa[:], wrtt[:, oh * CH:(oh + 1) * CH],
                                         a_col, start=True, stop=True)
                        nc.vector.scalar_tensor_tensor(
                            st[:, 0:1], wa[:], 1.0, pwbN(oh),
                            op0=MUL, op1=ADD)
                        qp = psB.tile([CH, K], f32, name=f"qp{oh}", tag="qp")
                        nc.tensor.matmul(qp[:], wrtt[:, oh * CH:(oh + 1) * CH],
                                         g_mat, start=True, stop=True)
                        scr = wb.tile([CH, K], f32, name="scr")
                        quad = wb.tile([CH, 1], f32, name=f"quad{oh}")
                        nc.vector.tensor_mul(scr[:], qp[:], wrt[oh][:])
                        nc.vector.reduce_sum(quad[:], scr[:],
                                             axis=mybir.AxisListType.X)
                        t2 = wb.tile([CH, 1], f32, name=f"t2{oh}")
                        nc.vector.scalar_tensor_tensor(
                            t2[:], wa[:], pwb2(oh), npwb2(oh),
                            op0=MUL, op1=ADD)
                        nc.vector.tensor_add(st[:, 1:2], t2[:], quad[:])
                        stats.append(st)
                    gs = wb.tile([16, 4], f32, name="gs")
                    for oh in range(2):
                        gp2 = psB.tile([16, 2], f32, name=f"gp2{oh}", tag="gp2")
                        nc.tensor.matmul(gp2[:], gind_ap(), stats[oh][:],
                                         start=True, stop=True)
                        nc.vector.tensor_copy(gs[:, oh * 2:(oh + 1) * 2], gp2[:])
                    gs4 = gs[:].rearrange("p (h c) -> p h c", h=2)
                    mv = wb.tile([16, 4], f32, name="mv")  # [mean, rstd] x half
                    mv4 = mv[:].rearrange("p (h c) -> p h c", h=2)
                    cinv = 1.0 / (GSZ * N)
                    nc.vector.tensor_scalar_mul(mv4[:, :, 0:1], gs4[:, :, 0:1],
                                                cinv)
                    ex2 = wb.tile([16, 2], f32, name="ex2")
                    nc.vector.tensor_scalar_mul(ex2[:], gs4[:, :, 1], cinv)
                    m2 = wb.tile([16, 2], f32, name="m2")
                    nc.vector.tensor_mul(m2[:], mv4[:, :, 0], mv4[:, :, 0])
                    var = wb.tile([16, 2], f32, name="var")
                    nc.vector.tensor_sub(var[:], ex2[:], m2[:])
                    epst = wb.tile([16, 1], f32, name="epst")
                    nc.vector.memset(epst[:], EPS)
                    std = wb.tile([16, 2], f32, name="std")
                    nc.scalar.activation(std[:], var[:], Sqrt, bias=epst[:])
                    nc.vector.reciprocal(mv4[:, :, 1], std[:])
                    lw = wb.tile([CH, C], f32r, name="lw")
                    for oh in range(2):
                        ep = psB.tile([CH, 2], f32, name=f"ep{oh}", tag="ep")
                        nc.tensor.matmul(ep[:], gindT_ap(),
                                         mv[:, oh * 2:(oh + 1) * 2],
                                         start=True, stop=True)
                        ach = wb.tile([CH, 1], f32, name=f"ach{oh}")
                        nc.vector.tensor_mul(ach[:], gng(oh), ep[:, 1:2])
                        cst = wb.tile([CH, 1], f32, name=f"cst{oh}")
                        nc.vector.tensor_sub(cst[:], pwb(oh), ep[:, 0:1])
                        nc.vector.tensor_mul(cst[:], ach[:], cst[:])
                        nc.vector.tensor_add(cst[:], gnb(oh), cst[:])
                        lwp = wb.tile([CH, K], f32, name=f"lwp{oh}")
                        nc.vector.tensor_scalar_mul(lwp[:], wrt[oh][:], ach[:])
                        # fold the per-channel const into lw: softmax columns
                        # sum to 1, so lw'[k,o] = lw[k,o] + const[o] adds
                        # const[o] to every output pixel
                        nc.vector.tensor_scalar_add(lwp[:], lwp[:], cst[:])
                        ltp = psB.tile([K, CH], f32, name=f"ltp{oh}", tag="ltp")
                        nc.tensor.transpose(ltp[:], lwp[:], identf())
                        nc.scalar.activation(lw[0:K, oh * CH:(oh + 1) * CH],
                                             ltp[:], Copy)
                        nc.scalar.activation(lw[K:CH, oh * CH:(oh + 1) * CH],
                                             ltp[:], Copy)

                # ---------- phase C ----------
                # both half-chunks staged into one osb tile -> one DMA per
                # chunk (halves the HWDGE generation load)
                out2 = out_d.rearrange("(u p) n -> p u n", u=2)
                with tc.tile_pool(name="psU", bufs=8, space="PSUM") as psU:
                    for ch in range(NCHUNK):
                        cs = ch * CHUNK
                        rh = 0 if ch < 16 else K
                        ac = (ch % 16) * CHUNK
                        osb = wk2.tile([CH, 2 * CHUNK], f32, name="osb",
                                       tag="osb")
                        for oh in range(2):
                            up = psU.tile([CH, CHUNK], f32, name="up", tag="up")
                            nc.tensor.matmul(up[:],
                                             lw[rh:rh + K, oh * CH:(oh + 1) * CH],
                                             attnT[rh:rh + K, ac:ac + CHUNK],
                                             start=True, stop=False)
                            nc.tensor.matmul(up[:], identr_ap(),
                                             xh[oh][:, cs:cs + CHUNK],
                                             start=False, stop=True)
                            dst = osb[:, oh * CHUNK:(oh + 1) * CHUNK]
                            if oh == 0:
                                nc.vector.tensor_copy(dst, up[:])
                            else:
                                nc.scalar.activation(dst, up[:], Copy)
                        nc.sync.dma_start(
                            out2[:, :, cs:cs + CHUNK],
                            osb[:].rearrange("p (u n) -> p u n", u=2))
    nc.compile()
    return nc


def host_prep(centers, dw_w, dw_b, pw_w, pw_b, gn_g, gn_b):
    col = lambda v: np.asarray(v, dtype=np.float32).reshape(C)
    # cpack [C, 376]: [0:64 cenT | 64:113 dwt | 113:120 cols | 120:376 pwT]
    cpack = np.zeros((C, 376), dtype=np.float32)
    cpack[:, 0:K] = (np.asarray(centers) * (C ** -0.5)).T.astype(np.float32)
    cpack[:, 64:113] = np.asarray(dw_w, dtype=np.float32).reshape(C, 49)
    pw_b = np.asarray(pw_b)
    for i, v in enumerate((dw_b, pw_b, gn_g, gn_b, pw_b * float(N),
                           pw_b * pw_b * float(N), 2.0 * pw_b)):
        cpack[:, 113 + i] = col(v)
    cpack[:, 120:376] = np.asarray(pw_w, dtype=np.float32).T
    # mpack [128, 466]: [ident | identr_hi | j64p | ones | gind | gindT]
    mpack = np.zeros((CH, 466), dtype=np.float32)
    mpack[:, 0:128] = np.eye(CH, dtype=np.float32)
    mpack[K:CH, 128:192] = np.eye(K, dtype=np.float32)
    mpack[0:K, 192 + K:320] = 1.0    # j64p: [0 | ones(64,64)]
    mpack[:, 320:322] = 1.0          # ones
    gind = np.zeros((CH, 16), dtype=np.float32)
    for c in range(CH):
        gind[c, c // GSZ] = 1.0
    mpack[:, 322:338] = gind
    mpack[0:16, 338:466] = gind.T
    return {"cpack": cpack, "mpack": mpack}


_NC_CACHE = {}


def _get_nc(repeat=1, sim_f32r=True, use_div=False):
    key = (repeat, sim_f32r, use_div)
    if key not in _NC_CACHE:
        _NC_CACHE[key] = build_nc(repeat=repeat, sim_f32r=sim_f32r,
                                  use_div=use_div)
    return _NC_CACHE[key]


def kernel(x, centers, dw_w, dw_b, pw_w, pw_b, gn_g, gn_b,
           repeat=1, sim_f32r=True, use_div=False):
    x = np.asarray(x)
    B = x.shape[0]
    nc = _get_nc(repeat=repeat, sim_f32r=sim_f32r, use_div=use_div)
    consts = host_prep(np.asarray(centers), np.asarray(dw_w), np.asarray(dw_b),
                       np.asarray(pw_w), np.asarray(pw_b),
                       np.asarray(gn_g), np.asarray(gn_b))
    in_maps = []
    for b in range(B):
        m = dict(consts)
        m["x"] = np.ascontiguousarray(x[b].reshape(C, N).astype(np.float32))
        in_maps.append(m)
    res = run_bass_kernel_spmd(nc, in_maps, core_ids=list(range(B)))
    out = np.stack([r["out"].reshape(C, 128, 128) for r in res.results])
    return out.astype(np.float32)
